# revision 13
# baseline (speedup 1.0000x reference)
"""Trainium2 Bass kernel for nn_Disentangler (segment_reduce).

Reference pipeline per timestamp t (T=8, NTOK=32768, D=128):
  xn = LayerNorm(x)                                 [NTOK, D]
  scatter xn rows into 65536-entry table (unique idx), zeros elsewhere
  per contiguous 8192-group: mean(gelu(row @ w1 + b1) @ w2 + b2)   -> comp [8, 64]
  compressed = LayerNorm(comp.reshape(t,1,512));  ortho loss over comp

Algebraic restructuring used here:
  * scatter + group-mean == segment-sum over tokens keyed by node_idx >> 13,
    plus a closed-form correction for the empty slots.
  * mean-subtraction of LN folds into the weights: w1'' = (I - 11^T/D) diag(ln1_w) w1
  * per-token inverse-std rs folds into the gelu activation's per-partition scale.
  * the trailing tiny ops (@w2, /8192, +b2, final LN, ortho) run on host from the
    device-computed H[t, l, e] = sum_{tok in group l} gelu(rs * (x @ w1'')).

Device work per core (one timestamp): stream 16.8 MB of x, bn_stats for LN
variance, transpose x tiles on the PE, one [*,128]x[128,128] bf16 matmul, gelu,
and a [tok,8]^T @ [tok,128] segment matmul accumulating in PSUM.

Sharding: data-parallel over T across the 8 NeuronCores (one timestamp per
core); the small weights are replicated; no collectives (host reduces the
scalar loss from the gathered outputs).
"""

import math
import os

import numpy as np
import ml_dtypes

import concourse.bass as bass
import concourse.mybir as mybir
import concourse.tile as tile
from concourse.bass_utils import run_bass_kernel_spmd
from concourse.masks import make_identity

BF16 = mybir.dt.bfloat16
F32 = mybir.dt.float32

T, NTOK, D = 8, 32768, 128
COMP_LEN, COMP_DIM = 8, 64
P = 128
TILES = NTOK // P  # 256
WAVE = 32          # tiles per wave
NWAVES = TILES // WAVE
EPS = 1e-5

# "pe": transpose via TensorE + DVE copy; "xbar": transpose via DMA xbar
TRANSPOSE_MODE = os.environ.get("SEG_TRANSPOSE_MODE", "pe")


# ---------------------------------------------------------------------------
# Workaround for a walrus codegen limit in this toolchain: an instruction
# supports at most ONE sync-wait command, but Tile freely attaches several.
# After scheduling, hoist extra waits onto same-engine nops inserted directly
# before the offending instruction (engine order is preserved, so semantics
# are identical).
_MAX_WAITS = 1
_fixup_counter = [0]


def _split_multi_waits(nc):
    def fix_block(bb):
        lst = bb.instructions
        out = []
        changed = False
        for inst in lst:
            for blk in getattr(inst, "blocks", None) or []:
                fix_block(blk)
            si = getattr(inst, "sync_info", None)
            waits = list(si.on_wait) if si is not None else []
            if len(waits) > _MAX_WAITS:
                extra, keep = waits[:-_MAX_WAITS], waits[-_MAX_WAITS:]
                for w in extra:
                    _fixup_counter[0] += 1
                    n = mybir.InstNoOp(
                        name=f"waitfix-{_fixup_counter[0]}", ins=[], outs=[]
                    )
                    n.engine = inst.engine
                    n.sync_info = mybir.SyncInfo(on_wait=[w], on_update=[])
                    out.append(n)
                inst.sync_info = mybir.SyncInfo(
                    on_wait=keep, on_update=list(si.on_update)
                )
                changed = True
            out.append(inst)
        if changed:
            bb.instructions = out

    for fn in nc.m.functions:
        for bb in fn.blocks:
            fix_block(bb)
# ---------------------------------------------------------------------------


def _build_bass(has_bias: bool):
    """Trace the per-core bass program. Inputs (per core):
      xt     [NTOK, D]  f32   one timestamp of x
      w1pp   [D, D]     bf16  (I - 11^T/D) diag(ln1_w) w1
      onehot [P, TILES*8] bf16  onehot[p, gi*8+l] = [group(token gi*128+p) == l]
      b1bc   [P, D]     f32   broadcast of (ln1_b @ w1 + b1)   (only if has_bias)
    Output:
      hsum   [8, D]     f32   H[l, e] = sum_{tok in group l} gelu(...)[e]
    """
    nc = bass.Bass()
    xt = nc.declare_dram_parameter("xt", [NTOK, D], F32, isOutput=False)
    w1pp = nc.declare_dram_parameter("w1pp", [D, D], BF16, isOutput=False)
    onehot = nc.declare_dram_parameter("onehot", [P, TILES * 8], BF16, isOutput=False)
    if has_bias:
        b1bc = nc.declare_dram_parameter("b1bc", [P, D], F32, isOutput=False)
    hsum = nc.declare_dram_parameter("hsum", [8, D], F32, isOutput=True)

    with tile.TileContext(nc) as tc:
        with (
            tc.tile_pool(name="const", bufs=1) as const,
            tc.tile_pool(name="xw", bufs=3) as xwp,
            tc.tile_pool(name="fin", bufs=2) as finp,
            tc.tile_pool(name="rsp", bufs=1) as rsp,
            tc.tile_pool(name="xts", bufs=4) as xtsp,
            tc.tile_pool(name="hs", bufs=6) as hsp,
            tc.tile_pool(name="ptr", bufs=3, space="PSUM") as ptr,
            tc.tile_pool(name="pm", bufs=3, space="PSUM") as pmp,
            tc.tile_pool(name="pacc", bufs=1, space="PSUM") as paccp,
        ):
            w1_sb = const.tile([D, D], BF16)
            nc.gpsimd.dma_start(out=w1_sb[:], in_=w1pp[:])
            oh_sb = const.tile([P, TILES * 8], BF16)
            nc.gpsimd.dma_start(out=oh_sb[:], in_=onehot[:])
            if has_bias:
                b1_sb = const.tile([P, D], F32)
                nc.gpsimd.dma_start(out=b1_sb[:], in_=b1bc[:])
            ident = const.tile([P, P], BF16)
            make_identity(nc, ident[:])
            eps_sb = const.tile([P, 1], F32)
            nc.vector.memset(eps_sb[:], EPS)

            rs_all = rsp.tile([P, TILES], F32)
            stats_all = rsp.tile([P, TILES, 6], F32)
            Hp = paccp.tile([8, D], F32)

            # Token -> (partition, tile) mapping is partition-major:
            # tile j, partition p holds token p*TILES + j, so each partition
            # reads one contiguous run per wave-DMA (the host-built onehot
            # uses the same mapping).
            xt_pjd = xt.rearrange("(p j) d -> p j d", p=P)
            for w in range(NWAVES):
                ws = slice(w * WAVE, (w + 1) * WAVE)
                # ---- load one wave of tokens, cast f32 -> bf16 in the DMA
                xw = xwp.tile([P, WAVE, D], BF16)
                nc.gpsimd.dma_start(
                    out=xw[:], in_=xt_pjd[:, w * WAVE:(w + 1) * WAVE, :]
                )

                # ---- phase A: LN stats per tile (bn_stats emits even/odd
                # partial stats; combined below with Chan's formula), then the
                # wave's rs = 1/sqrt(var+eps).
                for i in range(WAVE):
                    gi = w * WAVE + i
                    nc.vector.bn_stats(
                        out=stats_all[:, gi, :], in_=xw[:, i, :]
                    )
                me = stats_all[:, ws, 1]
                mo = stats_all[:, ws, 4]
                se = stats_all[:, ws, 2]
                so = stats_all[:, ws, 5]
                delta = finp.tile([P, WAVE], F32, tag="delta")
                d2 = finp.tile([P, WAVE], F32, tag="d2")
                ssum = finp.tile([P, WAVE], F32, tag="ssum")
                var128 = finp.tile([P, WAVE], F32, tag="var128")
                sd = finp.tile([P, WAVE], F32, tag="sd")
                nc.vector.tensor_tensor(
                    out=delta[:], in0=mo, in1=me, op=mybir.AluOpType.subtract
                )
                nc.vector.tensor_tensor(
                    out=d2[:], in0=delta[:], in1=delta[:], op=mybir.AluOpType.mult
                )
                nc.vector.tensor_tensor(
                    out=ssum[:], in0=se, in1=so, op=mybir.AluOpType.add
                )
                # var*128 = (var_e*64 + var_o*64) + 32*delta^2
                nc.vector.scalar_tensor_tensor(
                    out=var128[:], in0=d2[:], scalar=32.0, in1=ssum[:],
                    op0=mybir.AluOpType.mult, op1=mybir.AluOpType.add,
                )
                nc.scalar.activation(
                    out=sd[:], in_=var128[:],
                    func=mybir.ActivationFunctionType.Sqrt,
                    bias=eps_sb[:], scale=1.0 / 128.0,
                )
                nc.vector.reciprocal(out=rs_all[:, ws], in_=sd[:])

                # ---- phase B: per tile transpose -> mm1 -> gelu -> seg-mm
                for i2 in range(WAVE // 2):
                    xT2 = xtsp.tile([P, 2, P], BF16)
                    if TRANSPOSE_MODE == "pe":
                        tp = ptr.tile([P, 2, P], BF16)
                        for k in range(2):
                            nc.tensor.transpose(
                                out=tp[:, k, :], in_=xw[:, i2 * 2 + k, :],
                                identity=ident[:],
                            )
                        nc.vector.tensor_copy(out=xT2[:], in_=tp[:])
                    else:
                        nc.sync.dma_start_transpose(
                            out=xT2[:],
                            in_=xw[:, i2 * 2:(i2 + 1) * 2, :],
                        )
                    for k in range(2):
                        gi = w * WAVE + i2 * 2 + k
                        hp = pmp.tile([P, D], F32)
                        nc.tensor.matmul(
                            out=hp[:], lhsT=xT2[:, k, :], rhs=w1_sb[:],
                            start=True, stop=True,
                        )
                        h_s = hsp.tile([P, D], BF16)
                        if has_bias:
                            s1 = hsp.tile([P, D], F32, tag="s1")
                            nc.scalar.activation(
                                out=s1[:], in_=hp[:],
                                func=mybir.ActivationFunctionType.Copy,
                                scale=rs_all[:, gi:gi + 1],
                            )
                            nc.vector.tensor_tensor(
                                out=s1[:], in0=s1[:], in1=b1_sb[:],
                                op=mybir.AluOpType.add,
                            )
                            nc.scalar.activation(
                                out=h_s[:], in_=s1[:],
                                func=mybir.ActivationFunctionType.Gelu,
                            )
                        else:
                            nc.scalar.activation(
                                out=h_s[:], in_=hp[:],
                                func=mybir.ActivationFunctionType.Gelu,
                                scale=rs_all[:, gi:gi + 1],
                            )
                        nc.tensor.matmul(
                            out=Hp[:],
                            lhsT=oh_sb[:, gi * 8:(gi + 1) * 8],
                            rhs=h_s[:],
                            start=(gi == 0),
                            stop=(gi == TILES - 1),
                            skip_group_check=True,
                        )

            Hs = const.tile([8, D], F32)
            nc.vector.tensor_copy(out=Hs[:], in_=Hp[:])
            nc.gpsimd.dma_start(out=hsum[:], in_=Hs[:])

    _split_multi_waits(nc)
    return nc


_BASS_CACHE = {}


def _get_bass(has_bias: bool):
    key = (has_bias, TRANSPOSE_MODE)
    if key not in _BASS_CACHE:
        _BASS_CACHE[key] = _build_bass(has_bias)
    return _BASS_CACHE[key]


def _gelu_exact(v):
    v = np.asarray(v, np.float64)
    return 0.5 * v * (1.0 + np.vectorize(math.erf)(v / math.sqrt(2.0)))


def _prep_host(x, ln1_w, ln1_b, w1, b1, node_idx, padded_node_mask, n_entire):
    """Host-side folding and per-core input maps."""
    lw = np.asarray(ln1_w, np.float64)
    lb = np.asarray(ln1_b, np.float64)
    w1f = np.asarray(w1, np.float64)
    w1p = lw[:, None] * w1f
    w1pp = w1p - w1p.mean(axis=0, keepdims=True)  # fold mean subtraction
    b1pp = lb @ w1f + np.asarray(b1, np.float64)  # per-e bias
    has_bias = bool(np.max(np.abs(b1pp)) > 0)

    g = int(n_entire) // COMP_LEN
    shift = int(round(math.log2(g)))
    assert (1 << shift) == g

    w1pp_bf = w1pp.astype(np.float32).astype(ml_dtypes.bfloat16)
    in_maps = []
    counts = np.zeros((T, COMP_LEN), np.int64)
    mask = np.asarray(padded_node_mask)
    idx = np.asarray(node_idx)
    ar8 = np.arange(8, dtype=np.int64)
    for t in range(T):
        gid = (idx[t].astype(np.int64) >> shift)
        gid_eff = np.where(mask[t], gid, 8)
        counts[t] = np.bincount(gid_eff[gid_eff < 8], minlength=8)[:8]
        # partition-major token mapping: tile j / partition p <- token p*TILES+j
        oh = (gid_eff.reshape(P, TILES)[:, :, None] == ar8).astype(np.float32)
        m = {
            "xt": np.ascontiguousarray(x[t], dtype=np.float32),
            "w1pp": w1pp_bf,
            "onehot": np.ascontiguousarray(
                oh.reshape(P, TILES * 8).astype(ml_dtypes.bfloat16)
            ),
        }
        if has_bias:
            m["b1bc"] = np.broadcast_to(
                b1pp.astype(np.float32), (P, D)
            ).copy()
        in_maps.append(m)
    return in_maps, counts, b1pp, has_bias, g


def _epilogue(H, counts, g, b1, w2, b2, lnf_w, lnf_b):
    """comp -> final layernorm + ortho loss, all on host (tiny)."""
    H = np.asarray(H, np.float64)  # [T, 8, 128]
    h_empty = _gelu_exact(np.asarray(b1, np.float64))  # [128]
    w2f = np.asarray(w2, np.float64)
    b2f = np.asarray(b2, np.float64)
    hfull = H + (g - counts)[..., None] * h_empty[None, None, :]
    comp = hfull @ w2f / g + b2f  # [T, 8, 64]

    flat = comp.reshape(T, 1, COMP_LEN * COMP_DIM)
    mean = flat.mean(-1, keepdims=True)
    c = flat - mean
    var = (c * c).mean(-1, keepdims=True)
    compressed = c / np.sqrt(var + EPS) * np.asarray(lnf_w, np.float64) + np.asarray(
        lnf_b, np.float64
    )

    f = comp.transpose(1, 0, 2).reshape(COMP_LEN, -1)
    fn = f / np.linalg.norm(f, axis=-1, keepdims=True)
    dots = []
    for i in range(COMP_LEN - 1):
        for j in range(1, COMP_LEN):
            dots.append(np.sum(fn[i] * fn[j]) / np.sum(fn[i] + fn[j]))
    ortho = np.mean(np.asarray(dots) ** 2)
    return compressed.astype(np.float32), np.float32(ortho)


def kernel(x, ln1_w, ln1_b, w1, b1, w2, b2, lnf_w, lnf_b,
           padded_node_mask, node_idx, n_entire):
    in_maps, counts, b1pp, has_bias, g = _prep_host(
        x, ln1_w, ln1_b, w1, b1, node_idx, padded_node_mask, n_entire
    )
    nc = _get_bass(has_bias)
    res = run_bass_kernel_spmd(nc, in_maps, core_ids=list(range(T)))
    H = np.stack([res.results[t]["hsum"] for t in range(T)])  # [T, 8, 128]
    return _epilogue(H, counts, g, b1, w2, b2, lnf_w, lnf_b)


# revision 16
# speedup vs baseline: 7.3656x; 7.3656x over previous
"""Trainium2 Bass kernel for nn_Disentangler (segment_reduce).

Reference pipeline per timestamp t (T=8, NTOK=32768, D=128):
  xn = LayerNorm(x)                                 [NTOK, D]
  scatter xn rows into 65536-entry table (unique idx), zeros elsewhere
  per contiguous 8192-group: mean(gelu(row @ w1 + b1) @ w2 + b2)   -> comp [8, 64]
  compressed = LayerNorm(comp.reshape(t,1,512));  ortho loss over comp

Algebraic restructuring used here:
  * scatter + group-mean == segment-sum over tokens keyed by node_idx >> 13,
    plus a closed-form correction for the empty slots.
  * mean-subtraction of LN folds into the weights: w1'' = (I - 11^T/D) diag(ln1_w) w1
  * per-token inverse-std rs folds into the gelu activation's per-partition scale.
  * the trailing tiny ops (@w2, /8192, +b2, final LN, ortho) run on host from the
    device-computed H[t, l, e] = sum_{tok in group l} gelu(rs * (x @ w1'')).

Device work per core (one timestamp): stream 16.8 MB of x, bn_stats for LN
variance, transpose x tiles on the PE, one [*,128]x[128,128] bf16 matmul, gelu,
and a [tok,8]^T @ [tok,128] segment matmul accumulating in PSUM.

Sharding: data-parallel over T across the 8 NeuronCores (one timestamp per
core); the small weights are replicated; no collectives (host reduces the
scalar loss from the gathered outputs).
"""

import math
import os

import numpy as np
import ml_dtypes

import concourse.bass as bass
import concourse.mybir as mybir
import concourse.tile as tile
from concourse.bass_utils import run_bass_kernel_spmd
from concourse.masks import make_identity

BF16 = mybir.dt.bfloat16
F32 = mybir.dt.float32

T, NTOK, D = 8, 32768, 128
COMP_LEN, COMP_DIM = 8, 64
P = 128
TILES = NTOK // P  # 256
WAVE = 32          # tiles per wave
NWAVES = TILES // WAVE
EPS = 1e-5

# "pe": transpose via TensorE + DVE copy; "xbar": transpose via DMA xbar
TRANSPOSE_MODE = os.environ.get("SEG_TRANSPOSE_MODE", "pe")


# ---------------------------------------------------------------------------
# Workaround for a walrus codegen limit in this toolchain: an instruction
# supports at most ONE sync-wait command, but Tile freely attaches several.
# After scheduling, hoist extra waits onto same-engine nops inserted directly
# before the offending instruction (engine order is preserved, so semantics
# are identical).
_MAX_WAITS = 1
_fixup_counter = [0]


def _split_multi_waits(nc):
    def fix_block(bb):
        lst = bb.instructions
        out = []
        changed = False
        for inst in lst:
            for blk in getattr(inst, "blocks", None) or []:
                fix_block(blk)
            si = getattr(inst, "sync_info", None)
            waits = list(si.on_wait) if si is not None else []
            if len(waits) > _MAX_WAITS:
                extra, keep = waits[:-_MAX_WAITS], waits[-_MAX_WAITS:]
                for w in extra:
                    _fixup_counter[0] += 1
                    n = mybir.InstNoOp(
                        name=f"waitfix-{_fixup_counter[0]}", ins=[], outs=[]
                    )
                    n.engine = inst.engine
                    n.sync_info = mybir.SyncInfo(on_wait=[w], on_update=[])
                    out.append(n)
                inst.sync_info = mybir.SyncInfo(
                    on_wait=keep, on_update=list(si.on_update)
                )
                changed = True
            out.append(inst)
        if changed:
            bb.instructions = out

    for fn in nc.m.functions:
        for bb in fn.blocks:
            fix_block(bb)
# ---------------------------------------------------------------------------


def _build_bass(has_bias: bool, reps: int = 1):
    """Trace the per-core bass program. Inputs (per core):
      xt     [NTOK, D]  f32   one timestamp of x
      w1pp   [D, D]     bf16  (I - 11^T/D) diag(ln1_w) w1
      onehot [P, TILES*8] bf16  onehot[p, gi*8+l] = [group(token gi*128+p) == l]
      b1bc   [P, D]     f32   broadcast of (ln1_b @ w1 + b1)   (only if has_bias)
    Output:
      hsum   [8, D]     f32   H[l, e] = sum_{tok in group l} gelu(...)[e]
    """
    nc = bass.Bass()
    xt = nc.declare_dram_parameter("xt", [NTOK, D], F32, isOutput=False)
    w1pp = nc.declare_dram_parameter("w1pp", [D, D], BF16, isOutput=False)
    onehot = nc.declare_dram_parameter("onehot", [P, TILES * 8], BF16, isOutput=False)
    if has_bias:
        b1bc = nc.declare_dram_parameter("b1bc", [P, D], F32, isOutput=False)
    hsum = nc.declare_dram_parameter("hsum", [8, D], F32, isOutput=True)

    with tile.TileContext(nc) as tc:
        with (
            tc.tile_pool(name="const", bufs=1) as const,
            tc.tile_pool(name="xw", bufs=3) as xwp,
            tc.tile_pool(name="fin", bufs=2) as finp,
            tc.tile_pool(name="rsp", bufs=1) as rsp,
            tc.tile_pool(name="xts", bufs=4) as xtsp,
            tc.tile_pool(name="hs", bufs=6) as hsp,
            tc.tile_pool(name="ptr", bufs=3, space="PSUM") as ptr,
            tc.tile_pool(name="pm", bufs=3, space="PSUM") as pmp,
            tc.tile_pool(name="pacc", bufs=1, space="PSUM") as paccp,
        ):
            w1_sb = const.tile([D, D], BF16)
            nc.gpsimd.dma_start(out=w1_sb[:], in_=w1pp[:])
            oh_sb = const.tile([P, TILES * 8], BF16)
            nc.gpsimd.dma_start(out=oh_sb[:], in_=onehot[:])
            if has_bias:
                b1_sb = const.tile([P, D], F32)
                nc.gpsimd.dma_start(out=b1_sb[:], in_=b1bc[:])
            ident = const.tile([P, P], BF16)
            make_identity(nc, ident[:])
            eps_sb = const.tile([P, 1], F32)
            nc.vector.memset(eps_sb[:], EPS)

            rs_all = rsp.tile([P, TILES], F32)
            stats_all = rsp.tile([P, TILES, 6], F32)
            Hp = paccp.tile([8, D], F32)

            # Token -> (partition, tile) mapping is partition-major:
            # tile j, partition p holds token p*TILES + j, so each partition
            # reads one contiguous run per wave-DMA (the host-built onehot
            # uses the same mapping).
            xt_pjd = xt.rearrange("(p j) d -> p j d", p=P)
            for rep_w in range(reps * NWAVES):
                w = rep_w % NWAVES
                ws = slice(w * WAVE, (w + 1) * WAVE)
                # ---- load one wave of tokens, cast f32 -> bf16 in the DMA
                xw = xwp.tile([P, WAVE, D], BF16)
                nc.gpsimd.dma_start(
                    out=xw[:], in_=xt_pjd[:, w * WAVE:(w + 1) * WAVE, :]
                )

                # ---- phase A: LN stats per tile (bn_stats emits even/odd
                # partial stats; combined below with Chan's formula), then the
                # wave's rs = 1/sqrt(var+eps).
                for i in range(WAVE):
                    gi = w * WAVE + i
                    nc.vector.bn_stats(
                        out=stats_all[:, gi, :], in_=xw[:, i, :]
                    )
                me = stats_all[:, ws, 1]
                mo = stats_all[:, ws, 4]
                se = stats_all[:, ws, 2]
                so = stats_all[:, ws, 5]
                delta = finp.tile([P, WAVE], F32, tag="delta")
                d2 = finp.tile([P, WAVE], F32, tag="d2")
                ssum = finp.tile([P, WAVE], F32, tag="ssum")
                var128 = finp.tile([P, WAVE], F32, tag="var128")
                sd = finp.tile([P, WAVE], F32, tag="sd")
                nc.vector.tensor_tensor(
                    out=delta[:], in0=mo, in1=me, op=mybir.AluOpType.subtract
                )
                nc.vector.tensor_tensor(
                    out=d2[:], in0=delta[:], in1=delta[:], op=mybir.AluOpType.mult
                )
                nc.vector.tensor_tensor(
                    out=ssum[:], in0=se, in1=so, op=mybir.AluOpType.add
                )
                # var*128 = (var_e*64 + var_o*64) + 32*delta^2
                nc.vector.scalar_tensor_tensor(
                    out=var128[:], in0=d2[:], scalar=32.0, in1=ssum[:],
                    op0=mybir.AluOpType.mult, op1=mybir.AluOpType.add,
                )
                nc.scalar.activation(
                    out=sd[:], in_=var128[:],
                    func=mybir.ActivationFunctionType.Sqrt,
                    bias=eps_sb[:], scale=1.0 / 128.0,
                )
                nc.vector.reciprocal(out=rs_all[:, ws], in_=sd[:])

                # ---- phase B: per tile transpose -> mm1 -> gelu -> seg-mm
                for i2 in range(WAVE // 2):
                    xT2 = xtsp.tile([P, 2, P], BF16)
                    if TRANSPOSE_MODE == "pe":
                        tp = ptr.tile([P, 2, P], BF16)
                        for k in range(2):
                            nc.tensor.transpose(
                                out=tp[:, k, :], in_=xw[:, i2 * 2 + k, :],
                                identity=ident[:],
                            )
                        nc.vector.tensor_copy(out=xT2[:], in_=tp[:])
                    else:
                        nc.sync.dma_start_transpose(
                            out=xT2[:],
                            in_=xw[:, i2 * 2:(i2 + 1) * 2, :],
                        )
                    for k in range(2):
                        gi = w * WAVE + i2 * 2 + k
                        hp = pmp.tile([P, D], F32)
                        nc.tensor.matmul(
                            out=hp[:], lhsT=xT2[:, k, :], rhs=w1_sb[:],
                            start=True, stop=True,
                        )
                        h_s = hsp.tile([P, D], BF16)
                        if has_bias:
                            s1 = hsp.tile([P, D], F32, tag="s1")
                            nc.scalar.activation(
                                out=s1[:], in_=hp[:],
                                func=mybir.ActivationFunctionType.Copy,
                                scale=rs_all[:, gi:gi + 1],
                            )
                            nc.vector.tensor_tensor(
                                out=s1[:], in0=s1[:], in1=b1_sb[:],
                                op=mybir.AluOpType.add,
                            )
                            nc.scalar.activation(
                                out=h_s[:], in_=s1[:],
                                func=mybir.ActivationFunctionType.Gelu,
                            )
                        else:
                            nc.scalar.activation(
                                out=h_s[:], in_=hp[:],
                                func=mybir.ActivationFunctionType.Gelu,
                                scale=rs_all[:, gi:gi + 1],
                            )
                        nc.tensor.matmul(
                            out=Hp[:],
                            lhsT=oh_sb[:, gi * 8:(gi + 1) * 8],
                            rhs=h_s[:],
                            start=(gi == 0),
                            stop=(gi == TILES - 1),
                            skip_group_check=True,
                        )

            Hs = const.tile([8, D], F32)
            nc.vector.tensor_copy(out=Hs[:], in_=Hp[:])
            nc.gpsimd.dma_start(out=hsum[:], in_=Hs[:])

    _split_multi_waits(nc)
    return nc


_BASS_CACHE = {}


def _get_bass(has_bias: bool, reps: int = 1):
    key = (has_bias, TRANSPOSE_MODE, reps)
    if key not in _BASS_CACHE:
        _BASS_CACHE[key] = _build_bass(has_bias, reps)
    return _BASS_CACHE[key]


def _gelu_exact(v):
    v = np.asarray(v, np.float64)
    return 0.5 * v * (1.0 + np.vectorize(math.erf)(v / math.sqrt(2.0)))


def _prep_host(x, ln1_w, ln1_b, w1, b1, node_idx, padded_node_mask, n_entire):
    """Host-side folding and per-core input maps."""
    lw = np.asarray(ln1_w, np.float64)
    lb = np.asarray(ln1_b, np.float64)
    w1f = np.asarray(w1, np.float64)
    w1p = lw[:, None] * w1f
    w1pp = w1p - w1p.mean(axis=0, keepdims=True)  # fold mean subtraction
    b1pp = lb @ w1f + np.asarray(b1, np.float64)  # per-e bias
    has_bias = bool(np.max(np.abs(b1pp)) > 0)

    g = int(n_entire) // COMP_LEN
    shift = int(round(math.log2(g)))
    assert (1 << shift) == g

    w1pp_bf = w1pp.astype(np.float32).astype(ml_dtypes.bfloat16)
    in_maps = []
    counts = np.zeros((T, COMP_LEN), np.int64)
    mask = np.asarray(padded_node_mask)
    idx = np.asarray(node_idx)
    ar8 = np.arange(8, dtype=np.int64)
    for t in range(T):
        gid = (idx[t].astype(np.int64) >> shift)
        gid_eff = np.where(mask[t], gid, 8)
        counts[t] = np.bincount(gid_eff[gid_eff < 8], minlength=8)[:8]
        # partition-major token mapping: tile j / partition p <- token p*TILES+j
        oh = (gid_eff.reshape(P, TILES)[:, :, None] == ar8).astype(np.float32)
        m = {
            "xt": np.ascontiguousarray(x[t], dtype=np.float32),
            "w1pp": w1pp_bf,
            "onehot": np.ascontiguousarray(
                oh.reshape(P, TILES * 8).astype(ml_dtypes.bfloat16)
            ),
        }
        if has_bias:
            m["b1bc"] = np.broadcast_to(
                b1pp.astype(np.float32), (P, D)
            ).copy()
        in_maps.append(m)
    return in_maps, counts, b1pp, has_bias, g


def _epilogue(H, counts, g, b1, w2, b2, lnf_w, lnf_b):
    """comp -> final layernorm + ortho loss, all on host (tiny)."""
    H = np.asarray(H, np.float64)  # [T, 8, 128]
    h_empty = _gelu_exact(np.asarray(b1, np.float64))  # [128]
    w2f = np.asarray(w2, np.float64)
    b2f = np.asarray(b2, np.float64)
    hfull = H + (g - counts)[..., None] * h_empty[None, None, :]
    comp = hfull @ w2f / g + b2f  # [T, 8, 64]

    flat = comp.reshape(T, 1, COMP_LEN * COMP_DIM)
    mean = flat.mean(-1, keepdims=True)
    c = flat - mean
    var = (c * c).mean(-1, keepdims=True)
    compressed = c / np.sqrt(var + EPS) * np.asarray(lnf_w, np.float64) + np.asarray(
        lnf_b, np.float64
    )

    f = comp.transpose(1, 0, 2).reshape(COMP_LEN, -1)
    fn = f / np.linalg.norm(f, axis=-1, keepdims=True)
    dots = []
    for i in range(COMP_LEN - 1):
        for j in range(1, COMP_LEN):
            dots.append(np.sum(fn[i] * fn[j]) / np.sum(fn[i] + fn[j]))
    ortho = np.mean(np.asarray(dots) ** 2)
    return compressed.astype(np.float32), np.float32(ortho)


def kernel(x, ln1_w, ln1_b, w1, b1, w2, b2, lnf_w, lnf_b,
           padded_node_mask, node_idx, n_entire):
    in_maps, counts, b1pp, has_bias, g = _prep_host(
        x, ln1_w, ln1_b, w1, b1, node_idx, padded_node_mask, n_entire
    )
    nc = _get_bass(has_bias)
    res = run_bass_kernel_spmd(nc, in_maps, core_ids=list(range(T)))
    H = np.stack([res.results[t]["hsum"] for t in range(T)])  # [T, 8, 128]
    return _epilogue(H, counts, g, b1, w2, b2, lnf_w, lnf_b)
